# revision 1
# baseline (speedup 1.0000x reference)
"""Trainium2 Bass kernel: retrieval-kNN memory system.

Computation (see reference):
  sims = cosine(query, memory_keys[m])  for m in 0..65535
  idx  = top_32(sims); mem_summary = mean(memory_values[idx], axis=0)
  out  = fusion_w @ concat([core_output, study_output, mem_summary]) + fusion_b

Distribution over 8 NeuronCores:
  - memory_keys / memory_values row-sharded: 8192 rows per core.
  - Each core streams its key shard once, computing per-row dot(query, k)
    (DVE tensor_tensor_reduce) and per-row sum(k^2) (ACT Square+accum,
    in-place on the key tile) at HBM rate.
  - Local exact top-32 by score = dot * rsqrt(sumsq) via repeated
    max8/match_replace, merged across partitions through DRAM bounces.
  - AllGather of the 8x32 candidate values; every core redundantly reduces
    the 256 candidates to the global top-32 and its threshold tau.
  - Rows with score >= tau are located locally (mask * iota cascade),
    their memory_values rows gathered with a bounds-checked indirect DMA
    (not-owned slots OOB-skipped), summed via a ones-matmul and
    AllReduced -> 32*mem_summary everywhere.
  - fusion_w row-sharded (512 rows/core) and applied as three chained
    tensor_tensor_reduce matvecs (core | study | mem thirds; the mem third
    scaled by 1/32); fusion_b is the reduction seed.
"""

import sys

import numpy as np

try:
    import concourse.bass as _probe  # noqa: F401
except Exception:  # pragma: no cover
    sys.path.insert(0, "/opt/trn_rl_repo")

E = 4096
M = 65536
NCORES = 8
MS = M // NCORES  # 8192 key/value rows per core
TILES = MS // 128  # 64 streaming tiles
TOPK = 32
WROWS = E // NCORES  # 512 fusion output rows per core
RG = WROWS // 128  # 4 fusion row groups
NEG = -1.0e30

_CACHED_NC = None


def _top32_rounds(nc, work, cand, imm):
    """cand[:, 0:32] = descending top-32 of each partition row of `work`.

    Destroys `work` (found entries replaced with `imm`)."""
    for r in range(4):
        sl = cand[:, 8 * r : 8 * r + 8]
        nc.vector.max(out=sl, in_=work)
        nc.vector.match_replace(
            out=work, in_to_replace=sl, in_values=work, imm_value=imm
        )


def build_module():
    import concourse.bacc as bacc
    import concourse.bass as bass
    import concourse.mybir as mybir
    import concourse.tile as tile

    f32 = mybir.dt.float32
    i32 = mybir.dt.int32
    Alu = mybir.AluOpType
    Act = mybir.ActivationFunctionType
    groups = [list(range(NCORES))]

    nc = bacc.Bacc(
        "TRN2", target_bir_lowering=False, debug=False, num_devices=NCORES
    )

    keys = nc.declare_dram_parameter("keys", [MS, E], f32, isOutput=False)
    vals = nc.declare_dram_parameter("vals", [MS, E], f32, isOutput=False)
    q = nc.declare_dram_parameter("q", [128, E], f32, isOutput=False)
    co = nc.declare_dram_parameter("co", [128, E], f32, isOutput=False)
    so = nc.declare_dram_parameter("so", [128, E], f32, isOutput=False)
    onesrow = nc.declare_dram_parameter("onesrow", [1, 128], f32, isOutput=False)
    w12 = nc.declare_dram_parameter("w12", [WROWS, 2 * E], f32, isOutput=False)
    w3 = nc.declare_dram_parameter("w3", [WROWS, E], f32, isOutput=False)
    bias = nc.declare_dram_parameter("bias", [WROWS], f32, isOutput=False)
    iota_in = nc.declare_dram_parameter("iota", [128, TILES], f32, isOutput=False)
    out = nc.declare_dram_parameter("out", [WROWS], f32, isOutput=True)

    with tile.TileContext(nc) as tc:
        with (
            tc.tile_pool(name="keys", bufs=3) as kp,
            tc.tile_pool(name="wstream", bufs=3) as wp,
            tc.tile_pool(name="persist", bufs=1) as sp,
            tc.tile_pool(name="psum", bufs=2, space="PSUM") as pp,
            tc.tile_pool(name="dram", bufs=1, space="DRAM") as dp,
        ):
            # ---- persistent SBUF state ----
            qb = sp.tile([128, E], f32, tag="qb")  # query bcast
            cob = sp.tile([128, E], f32, tag="cob")  # core_output bcast
            sob = sp.tile([128, E], f32, tag="sob")  # study_output bcast
            memb = sp.tile([128, E], f32, tag="memb")  # 32*mem_summary bcast
            # product sink: [128,1] tile written through a broadcast AP
            dumpc = sp.tile([128, 1], f32, tag="dumpc")
            touch = sp.tile([128, 1], f32, tag="touch")
            dots = sp.tile([128, TILES], f32, tag="dots")
            norms = sp.tile([128, TILES], f32, tag="norms")
            scores = sp.tile([128, TILES], f32, tag="scores")
            work = sp.tile([128, TILES], f32, tag="work")
            cand = sp.tile([128, 32], f32, tag="cand")
            m8 = sp.tile([8, 512], f32, tag="m8")
            c8 = sp.tile([8, 32], f32, tag="c8")
            allv = sp.tile([1, 256], f32, tag="allv")
            winners = sp.tile([1, 32], f32, tag="winners")
            tau128 = sp.tile([128, 1], f32, tag="tau128")
            iotaf = sp.tile([128, TILES], f32, tag="iotaf")
            wmask = sp.tile([128, TILES], f32, tag="wmask")
            midx = sp.tile([128, TILES], f32, tag="midx")
            idx32 = sp.tile([1, 32], f32, tag="idx32")
            negm = sp.tile([1, 32], f32, tag="negm")
            idx_i = sp.tile([1, 32], i32, tag="idx_i")
            idxp = sp.tile([32, 1], i32, tag="idxp")
            gbuf = sp.tile([32, E], f32, tag="gbuf")
            ones32 = sp.tile([32, 1], f32, tag="ones32")
            ones_row = sp.tile([1, 128], f32, tag="ones_row")
            partial = sp.tile([1, E], f32, tag="partial")
            memrow = partial  # disjoint lifetimes: partial dies at the AllReduce
            fsum = sp.tile([128, 4 * RG], f32, tag="fsum")  # fusion partials
            y = sp.tile([128, RG], f32, tag="y")

            # ---- DRAM bounce buffers ----
            b_cand = dp.tile([128 * 32], f32, tag="b_cand")
            b_c8 = dp.tile([8 * 32], f32, tag="b_c8")
            ag_in = dp.tile([32], f32, tag="ag_in")
            ag_out = dp.tile([NCORES * 32], f32, tag="ag_out")
            b_idx = dp.tile([32], i32, tag="b_idx")
            ar_in = dp.tile([E], f32, tag="ar_in")
            ar_out = dp.tile([E], f32, tag="ar_out")

            # ---- broadcast loads (host pre-replicated; plain contiguous DMAs) ----
            nc.sync.dma_start(out=qb[:], in_=q[:])
            nc.sync.dma_start(out=cob[:], in_=co[:])
            nc.sync.dma_start(out=sob[:], in_=so[:])
            nc.sync.dma_start(out=ones_row[:], in_=onesrow[:])
            bias_v = bias[:].rearrange("(g p) -> g p", p=128)
            for g in range(RG):
                # bias lands in the 4th fusion-partial column of its group
                nc.scalar.dma_start(
                    out=fsum[:, 4 * g + 3 : 4 * g + 4], in_=bias_v[g][:, None]
                )
            nc.sync.dma_start(out=iotaf[:], in_=iota_in[:])
            nc.vector.memset(ones32[:], 1.0)
            nc.vector.memset(gbuf[:], 0.0)
            # absorb the broadcast-load DMA waits on cheap copies so later
            # compute instructions carry at most one sync wait each
            nc.vector.tensor_copy(out=touch[:], in_=qb[:, 0:1])
            nc.vector.tensor_copy(out=touch[:], in_=cob[:, 0:1])
            nc.vector.tensor_copy(out=touch[:], in_=sob[:, 0:1])
            # dummy matmul so the PE observes ones_row's DMA before its real work
            scrap_ps = pp.tile([128, 1], f32, tag="pcol")
            nc.tensor.matmul(
                out=scrap_ps[:],
                lhsT=ones_row[:],
                rhs=ones_row[0:1, 0:1],
                start=True,
                stop=True,
            )

            # ---- stream key shard: dots (DVE) + sum-of-squares (ACT) ----
            keys_v = keys[:].rearrange("(t p) e -> t p e", p=128)
            for t in range(TILES):
                kt = kp.tile([128, E], f32, tag="kt")
                nc.sync.dma_start(out=kt[:], in_=keys_v[t])
                # dots[:, t] = sum(kt * qb) along free axis (fused, one pass)
                nc.vector.scalar_tensor_tensor(
                    out=dumpc[:].broadcast_to([128, E]),
                    in0=kt[:],
                    scalar=1.0,
                    in1=qb[:],
                    op0=Alu.mult,
                    op1=Alu.mult,
                    accum_out=dots[:, t : t + 1],
                )
                # in-place square; destroys kt after the dot has read it
                nc.scalar.activation(
                    out=kt[:],
                    in_=kt[:],
                    func=Act.Square,
                    accum_out=norms[:, t : t + 1],
                )

            # ---- scores = dots * rsqrt(norms)  (ranking-equivalent to cosine) ----
            nc.scalar.activation(out=work[:], in_=norms[:], func=Act.Sqrt)
            nc.vector.reciprocal(out=work[:], in_=work[:])
            nc.vector.tensor_mul(out=scores[:], in0=dots[:], in1=work[:])

            # ---- local exact top-32 of 8192 scores ----
            nc.vector.tensor_copy(out=work[:], in_=scores[:])
            _top32_rounds(nc, work[:], cand[:], NEG)
            b_cand_v = b_cand[:].rearrange("(p c) -> p c", p=128)
            nc.scalar.dma_start(out=b_cand_v, in_=cand[:])
            nc.scalar.dma_start(
                out=m8[:], in_=b_cand[:].rearrange("(j f) -> j f", j=8)
            )
            _top32_rounds(nc, m8[:], c8[:], NEG)
            b_c8_v = b_c8[:].rearrange("(p c) -> p c", p=8)
            nc.scalar.dma_start(out=b_c8_v, in_=c8[:])
            nc.scalar.dma_start(
                out=allv[:], in_=b_c8[:].rearrange("(j f) -> j f", j=1)
            )
            _top32_rounds(nc, allv[:], winners[:], NEG)
            nc.scalar.dma_start(out=ag_in[None, :], in_=winners[:])

            # ---- all-gather candidates; global top-32 + threshold tau ----
            nc.gpsimd.collective_compute(
                "AllGather",
                Alu.bypass,
                replica_groups=groups,
                ins=[ag_in.opt()],
                outs=[ag_out.opt()],
            )
            nc.scalar.dma_start(
                out=allv[:], in_=ag_out[:].rearrange("(j f) -> j f", j=1)
            )
            _top32_rounds(nc, allv[:], winners[:], NEG)
            # tau128[p] = winners[31] via outer product ones_row^T @ tau
            tau_ps = pp.tile([128, 1], f32, tag="pcol")
            nc.tensor.matmul(
                out=tau_ps[:],
                lhsT=ones_row[:],
                rhs=winners[0:1, 31:32],
                start=True,
                stop=True,
            )
            nc.vector.tensor_copy(out=tau128[:], in_=tau_ps[:])

            # ---- locate this core's winning rows: mask -> indices ----
            nc.vector.tensor_scalar(
                out=wmask[:],
                in0=scores[:],
                scalar1=tau128[:, :1],
                scalar2=None,
                op0=Alu.is_ge,
            )
            nc.vector.tensor_mul(out=midx[:], in0=wmask[:], in1=iotaf[:])
            nc.vector.tensor_scalar_add(midx[:], midx[:], -1.0)
            _top32_rounds(nc, midx[:], cand[:], -1.0)
            nc.scalar.dma_start(out=b_cand_v, in_=cand[:])
            nc.scalar.dma_start(
                out=m8[:], in_=b_cand[:].rearrange("(j f) -> j f", j=8)
            )
            _top32_rounds(nc, m8[:], c8[:], -1.0)
            nc.scalar.dma_start(out=b_c8_v, in_=c8[:])
            nc.scalar.dma_start(
                out=allv[:], in_=b_c8[:].rearrange("(j f) -> j f", j=1)
            )
            _top32_rounds(nc, allv[:], idx32[:], -1.0)

            # padding (-1) -> positive OOB sentinel; keep sentinel*row_stride
            # well inside int32 so the descriptor offset math cannot wrap
            nc.vector.tensor_scalar(
                out=negm[:], in0=idx32[:], scalar1=0.0, scalar2=None, op0=Alu.is_lt
            )
            nc.vector.tensor_scalar_mul(negm[:], negm[:], 1.0e5)
            nc.vector.tensor_add(out=idx32[:], in0=idx32[:], in1=negm[:])
            nc.vector.tensor_copy(out=idx_i[:], in_=idx32[:])
            nc.scalar.dma_start(out=b_idx[None, :], in_=idx_i[:])
            nc.scalar.dma_start(
                out=idxp[:], in_=b_idx[:].rearrange("(p one) -> p one", one=1)
            )

            # ---- gather owned winner rows (OOB slots skipped), sum, AllReduce ----
            nc.gpsimd.indirect_dma_start(
                out=gbuf[:],
                out_offset=None,
                in_=vals[:],
                in_offset=bass.IndirectOffsetOnAxis(ap=idxp[:, :1], axis=0),
                bounds_check=MS - 1,
                oob_is_err=False,
            )
            for ch in range(E // 512):
                ps = pp.tile([1, 512], f32, tag="ps")
                nc.tensor.matmul(
                    out=ps[:],
                    lhsT=ones32[:, :1],
                    rhs=gbuf[:, 512 * ch : 512 * (ch + 1)],
                    start=True,
                    stop=True,
                )
                nc.vector.tensor_copy(
                    out=partial[:, 512 * ch : 512 * (ch + 1)], in_=ps[:]
                )
            nc.scalar.dma_start(out=ar_in[None, :], in_=partial[:])
            nc.gpsimd.collective_compute(
                "AllReduce",
                Alu.add,
                replica_groups=groups,
                ins=[ar_in.opt()],
                outs=[ar_out.opt()],
            )
            nc.sync.dma_start(out=memrow[:], in_=ar_out[None, :])
            # replicate memrow across partitions: ones_row^T @ memrow chunks
            for ch in range(E // 512):
                mb_ps = pp.tile([128, 512], f32, tag="pbig")
                nc.tensor.matmul(
                    out=mb_ps[:],
                    lhsT=ones_row[:],
                    rhs=memrow[0:1, 512 * ch : 512 * (ch + 1)],
                    start=True,
                    stop=True,
                )
                nc.vector.tensor_copy(
                    out=memb[:, 512 * ch : 512 * (ch + 1)], in_=mb_ps[:]
                )

            # ---- fusion: y = W1@co + W2@so + (1/32) W3@memsum + b ----
            w12_v = w12[:].rearrange("(g p) e -> g p e", p=128)
            w3_v = w3[:].rearrange("(g p) e -> g p e", p=128)
            for g in range(RG):
                wa = wp.tile([128, E], f32, tag="w")
                wb = wp.tile([128, E], f32, tag="w")
                wc = wp.tile([128, E], f32, tag="w")
                nc.sync.dma_start(out=wa[:], in_=w12_v[g][:, 0:E])
                nc.sync.dma_start(out=wb[:], in_=w12_v[g][:, E : 2 * E])
                nc.sync.dma_start(out=wc[:], in_=w3_v[g])
                # three fused matvec partials + bias col, then one 4-wide reduce
                steps = [(wa, cob, 1.0), (wb, sob, 1.0), (wc, memb, 1.0 / TOPK)]
                for k, (wt, vb, sc) in enumerate(steps):
                    nc.vector.scalar_tensor_tensor(
                        out=dumpc[:].broadcast_to([128, E]),
                        in0=wt[:],
                        scalar=sc,
                        in1=vb[:],
                        op0=Alu.mult,
                        op1=Alu.mult,
                        accum_out=fsum[:, 4 * g + k : 4 * g + k + 1],
                    )
                nc.vector.tensor_reduce(
                    out=y[:, g : g + 1],
                    in_=fsum[:, 4 * g : 4 * g + 4],
                    axis=mybir.AxisListType.X,
                    op=Alu.add,
                )
                nc.sync.dma_start(
                    out=out[128 * g : 128 * (g + 1)][:, None], in_=y[:, g : g + 1]
                )

    nc.compile()
    return nc


def get_module():
    global _CACHED_NC
    if _CACHED_NC is None:
        _CACHED_NC = build_module()
    return _CACHED_NC


def make_in_maps(
    core_output, study_output, query, memory_keys, memory_values, fusion_w, fusion_b
):
    f = np.float32
    # q/co/so are replicated across the 128 SBUF partitions host-side so the
    # device loads are plain contiguous DMAs (broadcast-view DMAs fan out
    # across many DMA queues and overflow instruction sync-wait slots)
    co = np.ascontiguousarray(
        np.broadcast_to(np.asarray(core_output, dtype=f), (128, E))
    )
    so = np.ascontiguousarray(
        np.broadcast_to(np.asarray(study_output, dtype=f), (128, E))
    )
    q = np.ascontiguousarray(np.broadcast_to(np.asarray(query, dtype=f), (128, E)))
    onesrow = np.ones((1, 128), dtype=f)
    # iota[p, t] = local row index (t*128 + p) + 1, as fp32
    iota = (
        np.arange(128, dtype=f)[:, None] + 128.0 * np.arange(TILES, dtype=f)[None, :]
    ) + 1.0
    in_maps = []
    for c in range(NCORES):
        rows = slice(c * MS, (c + 1) * MS)
        wr = slice(c * WROWS, (c + 1) * WROWS)
        in_maps.append(
            {
                "keys": np.ascontiguousarray(memory_keys[rows], dtype=f),
                "vals": np.ascontiguousarray(memory_values[rows], dtype=f),
                "q": q,
                "co": co,
                "so": so,
                "w12": np.ascontiguousarray(fusion_w[wr, : 2 * E], dtype=f),
                "w3": np.ascontiguousarray(fusion_w[wr, 2 * E :], dtype=f),
                "bias": np.ascontiguousarray(fusion_b[wr], dtype=f),
                "iota": iota,
                "onesrow": onesrow,
            }
        )
    return in_maps


def kernel(
    core_output,
    study_output,
    query,
    memory_keys,
    memory_values,
    fusion_w,
    fusion_b,
    top_k=TOPK,
    **_unused,
):
    assert int(top_k) == TOPK, f"kernel hardcodes top_k={TOPK}, got {top_k}"
    from concourse.bass_utils import run_bass_kernel_spmd

    nc = get_module()
    in_maps = make_in_maps(
        core_output, study_output, query, memory_keys, memory_values, fusion_w, fusion_b
    )
    res = run_bass_kernel_spmd(nc, in_maps, list(range(NCORES)))
    return np.concatenate([res.results[c]["out"] for c in range(NCORES)], axis=0)

